# revision 41
# baseline (speedup 1.0000x reference)
"""Trainium2 Bass kernel for block-diagonal (per-graph) multi-head attention.

Full inputs in, full output out. Host side: graphs (contiguous segments of
the sorted node dim) are assigned whole to 8 NeuronCores (2 padded slots per
core, boustrophedon by size), weights replicated, x pre-transposed; outputs
are gathered back and the foldable biases (bv via softmax-rows-sum-to-1, bo)
are applied on the host.

Device program (SPMD, one compiled program, per-core data):
  - projections in bf16 (Q^T, K^T feature-major as f32r; V node-major packed
    per head with an extra column fed from a per-core 0/1 "vones" vector --
    that column makes the AV matmul emit the softmax denominator as psum row
    64 while excluding padded keys exactly, so no masking is ever needed)
  - scores^T [keys, queries] per (slot, head) into a multi-bank psum tile,
    ONE merged exp per item on ACT (scale=1/8, no bias)
  - AV software-pipelined two items behind scores so the in-order PE never
    waits on ACT
  - normalization: reciprocal of the denominator row (f32r), PE-broadcast
    across 64 partitions, one DVE multiply into O^T; odd heads reach
    partition base 64 via an SBUF->SBUF DMA (compute engines cannot shift
    partitions)
  - per-slot phase overlap: pass1 of slot g runs concurrently with
    normalize of slot g-1; all output projections are deferred to a final
    phase where their inputs are already resolved
  - f32r everywhere on the attention path (even moving-free-dim >= 256
    required for the 1 cycle/row fast path -- small slots are padded up to
    256)
"""

import os
import sys

import ml_dtypes
import numpy as np

for _p in ("/opt/trn_rl_repo", os.path.expanduser("~/.axon_site/_ro/trn_rl_repo")):
    if os.path.isdir(_p) and _p not in sys.path:
        sys.path.insert(0, _p)

import concourse.bacc as bacc
import concourse.bass as bass
import concourse.mybir as mybir
import concourse.tile as tile
from concourse.bass_utils import run_bass_kernel_spmd

N_CORES = 8
HIDDEN = 512
NUM_HEADS = 8
HEAD_DIM = 64
KC = HIDDEN // 128  # contraction chunks of 128
F32 = mybir.dt.float32
F32R = mybir.dt.float32r
BF16 = mybir.dt.bfloat16
PROJ_BF16 = True  # bf16 DMA+projections (halves input bytes)
PAD_BIAS = -30.0  # exp(-30) ~ 1e-13: kills padded keys without inf/NaN risk
MAX_FREE = 512  # psum bank limit for fp32 free dim

_CACHE: dict = {}
_ABLATE: frozenset = frozenset()  # timing ablation: {"proj","attn","outproj"}


def _ceil_div(a, b):
    return -(-a // b)


def _split_free(n, max_w=MAX_FREE):
    """Split even n into nearly equal EVEN pieces <= max_w.

    f32r matmuls require an even moving free dim (HW probe: odd N fails
    walrus codegen), so every psum free width here is even.
    """
    assert n % 2 == 0, n
    h = n // 2
    k = _ceil_div(n, max_w)
    base = h // k
    rem = h - base * k
    out = []
    off = 0
    for i in range(k):
        w = 2 * (base + (1 if i < rem else 0))
        out.append((off, w))
        off += w
    return out


def _evict_qk(nc, out_t, ps, n_tiles, bias_t):
    # on ACT (scalar.add): the scalar engine is idle during the projection
    # window while DVE carries the V evictions
    widths = [w for (_, w) in n_tiles]
    total = sum(widths)
    if len(set(widths)) == 1:
        w = widths[0]
        nc.scalar.add(
            out=out_t[:, :total].rearrange("p (t w) -> p t w",
                                           t=len(n_tiles)),
            in_=ps[:, :, :w], add=bias_t[:])
    else:
        for i, (n0, nw) in enumerate(n_tiles):
            nc.scalar.add(
                out=out_t[:, n0:n0 + nw], in_=ps[:, i, :nw],
                add=bias_t[:])


def _build_program(slot_sizes):
    """Build + compile the SPMD Bass program for padded slot sizes."""
    G = len(slot_sizes)
    offs = [0]
    for s in slot_sizes:
        offs.append(offs[-1] + s)
    nc_tot = offs[-1]

    nc = bacc.Bacc("TRN2", target_bir_lowering=False, debug=False,
                   num_devices=N_CORES)

    DTP = BF16 if PROJ_BF16 else F32R  # projection operand dtype (x, Wq..Wv)
    xT = nc.dram_tensor("xT", [HIDDEN, nc_tot], DTP, kind="ExternalInput")
    wq = nc.dram_tensor("wq", [HIDDEN, HIDDEN], DTP, kind="ExternalInput")
    wk = nc.dram_tensor("wk", [HIDDEN, HIDDEN], DTP, kind="ExternalInput")
    wv = nc.dram_tensor("wv", [HIDDEN, HIDDEN], DTP, kind="ExternalInput")
    wo = nc.dram_tensor("wo", [HIDDEN, HIDDEN], F32, kind="ExternalInput")
    bqk = nc.dram_tensor("bqk", [2 * KC, 128], F32, kind="ExternalInput")
    # per-node 1.0 (real) / 0.0 (padded): becomes the "ones" column of the
    # packed V tiles, so padded keys are excluded from both the AV numerator
    # and the softmax denominator exactly — no exp bias needed.
    vones = nc.dram_tensor("vones", [nc_tot], F32R, kind="ExternalInput")
    y = nc.dram_tensor("y", [nc_tot, HIDDEN], F32, kind="ExternalOutput")

    n_tiles_all = _split_free(nc_tot)
    mchunks = {}
    for g in range(G):
        s = slot_sizes[g]
        mchunks[g] = [(mi * 128, min(128, s - mi * 128))
                      for mi in range(_ceil_div(s, 128))]
    # K columns are read in full-128 chunks by the scores matmuls so the
    # merged exp never sees unwritten psum rows; zero-pad the tail
    pad_tail = max(0, _ceil_div(slot_sizes[-1], 128) * 128 - slot_sizes[-1])
    nck = nc_tot + pad_tail

    # items and o65-supertile column layout
    items = []
    ocol = {}
    gcol = {}
    oc = 0
    for g in range(G):
        gcol[g] = oc
        for h in range(NUM_HEADS):
            ocol[(g, h)] = oc
            oc += slot_sizes[g]
            for (n0, nw) in _split_free(slot_sizes[g]):
                items.append((g, h, n0, nw))
    o_total = oc
    mg_max = max(len(mchunks[g]) for g in range(G))

    with tile.TileContext(nc) as tc:
        with (
            tc.tile_pool(name="persist", bufs=1) as pp,
            tc.tile_pool(name="ework", bufs=4) as ep,
            tc.tile_pool(name="vwork", bufs=3) as vp,
        ):
            # ---- persistent tiles (packed: one DMA per logical tensor) ---
            xt_all = pp.tile([128, KC, nc_tot], DTP, tag="xt", name="xt")
            wq_all = pp.tile([128, KC, HIDDEN], DTP, tag="wq", name="wq_t")
            wk_all = pp.tile([128, KC, HIDDEN], DTP, tag="wk", name="wk_t")
            wv_all = pp.tile([128, KC, HIDDEN], DTP, tag="wv", name="wv_t")
            wo_all = pp.tile([128, KC, HIDDEN], F32R, tag="wo", name="wo_t")
            bqk_t = pp.tile([128, 2 * KC], F32, tag="bqk", name="bqk_t")
            xt = [xt_all[:, c] for c in range(KC)]
            wqs = [wq_all[:, c] for c in range(KC)]
            wks = [wk_all[:, c] for c in range(KC)]
            wvs = [wv_all[:, c] for c in range(KC)]
            wos = [wo_all[:, c] for c in range(KC)]
            bq_t = [bqk_t[:, c:c + 1] for c in range(KC)]
            bk_t = [bqk_t[:, KC + c:KC + c + 1] for c in range(KC)]

            qts = [pp.tile([128, nc_tot], F32R, tag=f"qts{c}", name=f"qts{c}")
                   for c in range(KC)]
            kts = [pp.tile([128, nck], F32R, tag=f"kts{c}", name=f"kts{c}")
                   for c in range(KC)]
            ots = [pp.tile([128, nc_tot], F32R, tag=f"ots{c}", name=f"ots{c}")
                   for c in range(KC)]
            otmp = [pp.tile([64, nc_tot], F32R, tag=f"otmp{c}",
                            name=f"otmp{c}") for c in range(KC)]
            ones_t = pp.tile([128, HEAD_DIM], F32R, tag="ones", name="ones")
            v65 = {}
            for g in range(G):
                for mi, (m0, pm) in enumerate(mchunks[g]):
                    v65[(g, mi)] = pp.tile([128, NUM_HEADS, HEAD_DIM + 1],
                                           F32R, tag=f"v{g}_{mi}",
                                           name=f"v{g}_{mi}")
            # unnormalized O~ (row 64 = denominator) for ALL items, one tile
            o65all = pp.tile([65, o_total], F32, tag="o65", name="o65all")
            d_all = pp.tile([65, o_total], F32R, tag="dall", name="d_all")

            # ---- input DMAs (consumption order; packed) ---------------
            nc.gpsimd.dma_start(out=bqk_t[:],
                                in_=bqk[:].rearrange("b p -> p b"))
            for c in range(KC):
                nc.sync.dma_start(out=xt_all[:, c],
                                  in_=xT[c * 128:(c + 1) * 128, :])
                nc.sync.dma_start(out=wq_all[:, c],
                                  in_=wq[c * 128:(c + 1) * 128, :])
                nc.sync.dma_start(out=wk_all[:, c],
                                  in_=wk[c * 128:(c + 1) * 128, :])
            nc.sync.dma_start(out=wv_all[:],
                              in_=wv[:, :].rearrange("(c p) n -> p c n",
                                                     p=128))
            for g in range(G):
                for mi, (m0, pm) in enumerate(mchunks[g]):
                    src = vones[offs[g] + m0: offs[g] + m0 + pm, None, None]
                    src = bass.AP(tensor=src.tensor, offset=src.offset,
                                  ap=[src.ap[0], [0, NUM_HEADS], [1, 1]])
                    nc.gpsimd.dma_start(
                        out=v65[(g, mi)][:pm, :, HEAD_DIM:],
                        in_=src)
            nc.sync.dma_start(out=wo_all[:],
                              in_=wo[:, :].rearrange("(c p) n -> p c n",
                                                     p=128).bitcast(F32R))
            nc.gpsimd.memset(ones_t[:].bitcast(F32), 1.0)
            warm = vp.tile([1, 2], F32, tag="warm")
            nc.scalar.activation(out=warm[:], in_=ones_t[0:1, 0:2].bitcast(F32),
                                 func=mybir.ActivationFunctionType.Exp,
                                 scale=0.125)
            if pad_tail:
                for c in range(KC):
                    nc.gpsimd.memset(
                        kts[c][:, nc_tot:].bitcast(F32), 0.0)

            # ---- QK projections (phase-scoped PSUM) -------------------
            with tc.tile_pool(name="ps_qk", bufs=2, space="PSUM") as ps_qk:
                nt_n = len(n_tiles_all)

                def emit_qk(dc):
                    q_ps = ps_qk.tile([128, nt_n, MAX_FREE], F32, tag="qk")
                    for i, (n0, nw) in enumerate(n_tiles_all):
                        for c in range(KC):
                            nc.tensor.matmul(
                                q_ps[:, i, :nw],
                                wqs[c][:, dc * 128:(dc + 1) * 128],
                                xt[c][:, n0:n0 + nw],
                                start=(c == 0), stop=(c == KC - 1))
                    _evict_qk(nc, qts[dc], q_ps, n_tiles_all, bq_t[dc])
                    k_ps = ps_qk.tile([128, nt_n, MAX_FREE], F32, tag="qk")
                    for i, (n0, nw) in enumerate(n_tiles_all):
                        for c in range(KC):
                            nc.tensor.matmul(
                                k_ps[:, i, :nw],
                                wks[c][:, dc * 128:(dc + 1) * 128],
                                xt[c][:, n0:n0 + nw],
                                start=(c == 0), stop=(c == KC - 1))
                    _evict_qk(nc, kts[dc], k_ps, n_tiles_all, bk_t[dc])

                if "proj" not in _ABLATE:
                    for dc in range(KC):
                        emit_qk(dc)

            if "attn" in _ABLATE:
                items_run = []
            else:
                items_run = items

            # ---- attention: per-slot phases with cross-slot overlap ----
            # phase(g) = pass1 of slot g (scores -> merged exp -> AV),
            # interleaved (when the 8 PSUM banks allow) with pass2 +
            # output projection of slot g-1.
            def emit_v(g, mi, ps_o):
                m0, pm = mchunks[g][mi]
                a0 = offs[g] + m0
                v_ps = ps_o.tile([128, MAX_FREE], F32, tag="ops")
                for c in range(KC):
                    nc.tensor.matmul(
                        v_ps[:pm, :],
                        xt[c][:, a0:a0 + pm],
                        wvs[c][:],
                        start=(c == 0), stop=(c == KC - 1))
                nc.vector.tensor_copy(
                    out=v65[(g, mi)][:pm, :, :HEAD_DIM],
                    in_=v_ps[:pm, :].rearrange("p (h d) -> p h d",
                                               h=NUM_HEADS))

            def emit_scores(it, ps_s):
                g, h, n0, nw = it
                dc, r0, g0 = h // 2, (h % 2) * 64, offs[g]
                mg = len(mchunks[g])
                s_ps = ps_s.tile([128, mg, MAX_FREE], F32, tag=f"sps{g}")
                for mi, (m0, pm) in enumerate(mchunks[g]):
                    nc.tensor.matmul(
                        s_ps[:, mi, :nw],
                        kts[dc][r0:r0 + 64, g0 + m0:g0 + m0 + 128],
                        qts[dc][r0:r0 + 64, g0 + n0:g0 + n0 + nw],
                        start=True, stop=True)
                e_t = ep.tile([128, mg_max, MAX_FREE], F32R, tag="e")
                nc.scalar.activation(
                    out=e_t[:, :mg, :nw], in_=s_ps[:, :mg, :nw],
                    func=mybir.ActivationFunctionType.Exp,
                    scale=0.125)
                return e_t

            def emit_av(it, e_t, ps_o):
                g, h, n0, nw = it
                o_ps = ps_o.tile([65, MAX_FREE], F32, tag="ops")
                for mi, (m0, pm) in enumerate(mchunks[g]):
                    nc.tensor.matmul(
                        o_ps[:, :nw],
                        v65[(g, mi)][:pm, h, :],
                        e_t[:pm, mi, :nw],
                        start=(mi == 0), stop=(mi == len(mchunks[g]) - 1))
                c0 = ocol[(g, h)] + n0
                if len(mchunks[g]) < mg_max and h % 2 == 0:
                    nc.scalar.copy(out=o65all[:, c0:c0 + nw],
                                   in_=o_ps[:, :nw])
                else:
                    nc.vector.tensor_copy(out=o65all[:, c0:c0 + nw],
                                          in_=o_ps[:, :nw])

            def emit_norm(it, ps_rb):
                g, h, n0, nw = it
                dc, g0 = h // 2, offs[g]
                c0 = ocol[(g, h)] + n0
                with nc.allow_low_precision(
                        reason="f32r rounding for PE broadcast"):
                    nc.vector.reciprocal(
                        out=d_all[64:65, c0:c0 + nw],
                        in_=o65all[64:65, c0:c0 + nw])
                rb_ps = ps_rb.tile([64, MAX_FREE], F32, tag="rb")
                nc.tensor.matmul(
                    rb_ps[:, :nw],
                    ones_t[64:65, :],
                    d_all[64:65, c0:c0 + nw],
                    start=True, stop=True)
                if h % 2 == 0:
                    nc.vector.tensor_mul(
                        ots[dc][0:64, g0 + n0:g0 + n0 + nw],
                        o65all[0:64, c0:c0 + nw], rb_ps[:, :nw])
                else:
                    nc.vector.tensor_mul(
                        otmp[dc][:, g0 + n0:g0 + n0 + nw],
                        o65all[0:64, c0:c0 + nw], rb_ps[:, :nw])

            def emit_shift(g, dc):
                # partition shift (base 0 -> 64) needs DMA
                g0, gw = offs[g], slot_sizes[g]
                nc.sync.dma_start(out=ots[dc][64:128, g0:g0 + gw],
                                  in_=otmp[dc][:, g0:g0 + gw])

            def emit_yproj(g, m0, pm, ps_y):
                a0 = offs[g] + m0
                y_ps = ps_y.tile([128, MAX_FREE], F32, tag="y")
                for dc in range(KC):
                    nc.tensor.matmul(
                        y_ps[:pm, :],
                        ots[dc][:, a0:a0 + pm],
                        wos[dc][:],
                        start=(dc == 0), stop=(dc == KC - 1))
                y_sb = vp.tile([128, HIDDEN], F32, tag="ysb")
                nc.scalar.copy(out=y_sb[:pm, :], in_=y_ps[:pm, :])
                nc.sync.dma_start(out=y[a0:a0 + pm, :], in_=y_sb[:pm, :])

            def norm_thunks(g, ps_rb):
                out = [lambda it=it: emit_norm(it, ps_rb)
                       for it in items_run if it[0] == g]
                out += [lambda dc=dc: emit_shift(g, dc) for dc in range(KC)]
                return out

            def yproj_thunks(g, ps_y):
                if "outproj" in _ABLATE:
                    return []
                return [lambda m0=m0, pm=pm: emit_yproj(g, m0, pm, ps_y)
                        for (m0, pm) in mchunks[g]]

            def run_pass1(g, ps_s, ps_o, pre, extra):
                my_items = [i for i in items_run if i[0] == g]
                for th in pre:  # e.g. this slot's V projection
                    th()
                ei = 0
                per = (_ceil_div(len(extra), len(my_items))
                       if my_items else len(extra))
                pend = []
                for it in my_items:
                    e_t = emit_scores(it, ps_s)
                    pend.append((it, e_t))
                    if len(pend) > 2:
                        emit_av(*pend.pop(0), ps_o)
                    for _ in range(per):
                        if ei < len(extra):
                            extra[ei]()
                            ei += 1
                for p in pend:
                    emit_av(*p, ps_o)
                while ei < len(extra):
                    extra[ei]()
                    ei += 1

            if items_run:
                deferred_yproj = []  # slots normalized but not yet projected
                prev_g = None
                for g in range(G):
                    mg = len(mchunks[g])
                    can_overlap = (prev_g is not None
                                   and 2 * mg + 2 + 1 + 1 <= 8)
                    if prev_g is not None and not can_overlap:
                        with tc.tile_pool(name=f"ps_rbx{prev_g}", bufs=2,
                                          space="PSUM") as ps_rb:
                            for th in norm_thunks(prev_g, ps_rb):
                                th()
                        deferred_yproj.append(prev_g)
                        prev_g = None
                    s_bufs = 2 if 2 * mg + 2 <= 8 else 1
                    assert mg * s_bufs + 2 <= 8, (
                        f"slot {g} too large for PSUM: {mg} score banks")
                    with (
                        tc.tile_pool(name=f"ps_s{g}", bufs=s_bufs,
                                     space="PSUM") as ps_s,
                        tc.tile_pool(name=f"ps_o{g}", bufs=2,
                                     space="PSUM") as ps_o,
                    ):
                        pre = ([] if "proj" in _ABLATE else
                               [lambda mi=mi: emit_v(g, mi, ps_o)
                                for mi in range(mg)])
                        if prev_g is not None:
                            with (
                                tc.tile_pool(name=f"ps_rb{prev_g}", bufs=1,
                                             space="PSUM") as ps_rb,
                                tc.tile_pool(name=f"ps_y{prev_g}", bufs=1,
                                             space="PSUM") as ps_y,
                            ):
                                run_pass1(g, ps_s, ps_o, pre,
                                          norm_thunks(prev_g, ps_rb)
                                          + yproj_thunks(prev_g, ps_y))
                        else:
                            run_pass1(g, ps_s, ps_o, pre, [])
                    prev_g = g
                # final phase: remaining normalize + output projections
                with (
                    tc.tile_pool(name="ps_rbf", bufs=2,
                                 space="PSUM") as ps_rb,
                    tc.tile_pool(name="ps_yf", bufs=2,
                                 space="PSUM") as ps_y,
                ):
                    ready = []
                    for gd in deferred_yproj:
                        ready += yproj_thunks(gd, ps_y)
                    tail = (norm_thunks(prev_g, ps_rb)
                            if prev_g is not None else [])
                    tail += (yproj_thunks(prev_g, ps_y)
                             if prev_g is not None else [])
                    # interleave ready yprojs with the last slot's norms
                    i = j = 0
                    while i < len(ready) or j < len(tail):
                        if j < len(tail):
                            tail[j]()
                            j += 1
                        if i < len(ready):
                            ready[i]()
                            i += 1

    nc.compile()
    return nc


def _plan(batch):
    """Assign whole graphs (contiguous segments) to cores/slots.

    Returns (slot_sizes, assign) where assign[core][slot] = (start, size)
    of the graph segment in the global node order (size 0 = empty slot).
    """
    batch = np.asarray(batch)
    n = batch.shape[0]
    vals, starts, counts = np.unique(batch, return_index=True,
                                     return_counts=True)
    segs = sorted(zip(starts.tolist(), counts.tolist()),
                  key=lambda t: -t[1])
    n_slots = _ceil_div(len(segs), N_CORES)
    while len(segs) < n_slots * N_CORES:
        segs.append((0, 0))
    assign = [[None] * n_slots for _ in range(N_CORES)]
    slot_sizes = []
    for j in range(n_slots):
        block = segs[j * N_CORES:(j + 1) * N_CORES]
        order = range(N_CORES) if j % 2 == 0 else range(N_CORES - 1, -1, -1)
        for c, k in zip(order, range(N_CORES)):
            assign[c][j] = block[k]
        m = max(sz for (_, sz) in block)
        m = _ceil_div(m, 4) * 4
        # f32r matmuls drop to 4 cycles/row below a 256-wide moving free
        # dim — pad mid-sized slots up to 256 to stay on the fast path
        if m >= 64:
            m = max(m, 256)
        slot_sizes.append(m)
    # drop empty slots
    keep = [j for j, s in enumerate(slot_sizes) if s > 0]
    slot_sizes = [slot_sizes[j] for j in keep]
    assign = [[assign[c][j] for j in keep] for c in range(N_CORES)]
    return tuple(slot_sizes), assign


def kernel(x, batch, Wq, bq, Wk, bk, Wv, bv, Wo, bo):
    out, _ = _execute(dict(x=x, batch=batch, Wq=Wq, bq=bq, Wk=Wk, bk=bk,
                           Wv=Wv, bv=bv, Wo=Wo, bo=bo))
    return out


def _prepare(inputs):
    x = np.ascontiguousarray(np.asarray(inputs["x"], dtype=np.float32))
    Wq = np.asarray(inputs["Wq"], dtype=np.float32)
    Wk = np.asarray(inputs["Wk"], dtype=np.float32)
    Wv = np.asarray(inputs["Wv"], dtype=np.float32)
    Wo = np.asarray(inputs["Wo"], dtype=np.float32)
    bq = np.asarray(inputs["bq"], dtype=np.float32)
    bk = np.asarray(inputs["bk"], dtype=np.float32)
    bv = np.asarray(inputs["bv"], dtype=np.float32)
    bo = np.asarray(inputs["bo"], dtype=np.float32)

    slot_sizes, assign = _plan(inputs["batch"])
    offs = np.concatenate([[0], np.cumsum(slot_sizes)]).astype(int)
    nc_tot = int(offs[-1])

    dtp = ml_dtypes.bfloat16 if PROJ_BF16 else np.float32
    wqT = np.ascontiguousarray(Wq.T).astype(dtp)
    wkT = np.ascontiguousarray(Wk.T).astype(dtp)
    wvT = np.ascontiguousarray(Wv.T).astype(dtp)
    woT = np.ascontiguousarray(Wo.T)
    # V-bias and out-bias fold: softmax rows sum to 1, so attn@(V+bv) =
    # attn@V + bv, and (O+bv)@Wo.T + bo = O@Wo.T + (Wo@bv + bo).
    b_out = (Wo @ bv + bo).astype(np.float32)
    bqk = np.concatenate([bq.reshape(KC, 128), bk.reshape(KC, 128)], axis=0)
    bqk = np.ascontiguousarray(bqk)

    in_maps = []
    for c in range(N_CORES):
        xT = np.zeros((HIDDEN, nc_tot), dtype=np.float32)
        von = np.zeros((nc_tot,), dtype=np.float32)
        for j, (st, sz) in enumerate(assign[c]):
            if sz:
                xT[:, offs[j]:offs[j] + sz] = x[st:st + sz].T
                von[offs[j]:offs[j] + sz] = 1.0
        in_maps.append({
            "xT": xT.astype(dtp), "wq": wqT, "wk": wkT, "wv": wvT, "wo": woT,
            "bqk": bqk, "vones": von,
        })
    return slot_sizes, assign, offs, in_maps, b_out


def _gather(results, assign, offs, n_nodes, b_out):
    out = np.empty((n_nodes, HIDDEN), dtype=np.float32)
    for c in range(N_CORES):
        yc = results[c]["y"]
        for j, (st, sz) in enumerate(assign[c]):
            if sz:
                out[st:st + sz] = yc[offs[j]:offs[j] + sz]
    out += b_out[None, :]
    return out


def _execute(inputs, trace=False, **run_kwargs):
    slot_sizes, assign, offs, in_maps, b_out = _prepare(inputs)
    if slot_sizes not in _CACHE:
        _CACHE[slot_sizes] = _build_program(list(slot_sizes))
    nc = _CACHE[slot_sizes]
    res = run_bass_kernel_spmd(nc, in_maps, list(range(N_CORES)),
                               trace=trace, **run_kwargs)
    out = _gather(res.results, assign, offs,
                  np.asarray(inputs["x"]).shape[0], b_out)
    return out, res
